# revision 16
# baseline (speedup 1.0000x reference)
"""Trainium2 Bass kernel for nn_CNNConcatLinear (B=1024, N=24, PD=2, C=512).

Strategy: pure data-parallel over batch (128 per core x 8 cores).

Algebraic restructure of the conv stack: the conv input X is rank-4 per
batch sample (2 x-dims + c1-bias, all gated, plus the ungated hyper-bias),
so instead of contracting 1024 channels x 384 positions per chunk, we:

  phase A: new_ctx via the exact 3x3 softmax fold (as before).
  phase B: all CSL gates/hyper-biases as [feature, batch] matmuls (bf16).
  phase E (per 16x2-batch pair): effective per-batch tap weights
      E_T[(half,j',b^), co] = sum_ci G1W1ext[ci,(half,j',b^)] * W_delta[ci,co]
    where G1W1ext packs c1_w*g1 (j'<3) and the hyper-bias b1 (j'=3),
    and W_delta are the conv taps in a suffix-run co layout (runs are
    contiguous because tap sets nest: co>=RUN0[delta]).
  apply (per 16-batch chunk): trans[co, b, n] = sum_(delta,j',b^)
      E_T[...] * xim[...] against a host-built block-diagonal im2col of x
    (plus edge-mask rows for the bias term); positional encoding and conv
    biases are host-folded into the c3 bias table.
  then c3/c4/cl CSL layers exactly as before (f32r matmuls, FD=384).

bf16 is used for all matmuls whose free dim is < 256 (f32r costs 4
cycles/row there) and for the E/apply path; c3/c4/cl stay f32r.
"""

import math
import os

import numpy as np
import ml_dtypes

B, N, PD, C = 1024, 24, 2, 512
F = 2 * C
NCORES = 8
BLOC = B // NCORES          # 128 batch per core
BC = 16                     # batch chunk
NBC = BLOC // BC            # 8 chunks
NPAIR = NBC // 2            # 4 chunk-pairs
FREE = BC * N               # 384

# Tap structure: tap sets nest (each conv's taps are a prefix of
# [0, +-1, ..., +-5]); co-runs for |delta|=kappa start at RUN0K[kappa-1].
RUN0K = [512, 768, 832, 896, 960]           # run start for kappa=1..5
WK = [1024 - r for r in RUN0K]              # run width: 512,256,192,128,64
PAIRO = np.concatenate([[0], np.cumsum(WK)]).astype(int)  # offsets in pair tiles
PTOT = int(PAIRO[-1])                       # 1152
# convd column layout: [delta0 (1024) | -1,+1 | -2,+2 | ... | -5,+5]
CO_N = [1024 + 2 * int(PAIRO[k]) for k in range(5)]      # neg slice start
CO_P = [CO_N[k] + WK[k] for k in range(5)]               # pos slice start
ETOT = 1024 + 2 * PTOT                      # 3328

LAST_RESULTS = None         # BassKernelResults from the most recent run


def _pe_table():
    pos = np.arange(N, dtype=np.float32)[:, None]
    div = np.exp(np.arange(0, F, 2, dtype=np.float32) * (-np.log(10000.0) / F))
    pe = np.zeros((N, F), dtype=np.float32)
    pe[:, 0::2] = np.sin(pos * div)
    pe[:, 1::2] = np.cos(pos * div)
    return pe


def _f32(a):
    return np.ascontiguousarray(np.asarray(a, dtype=np.float32))


def _bf16(a):
    return np.ascontiguousarray(np.asarray(a, dtype=ml_dtypes.bfloat16))


def _build(host, num_devices=NCORES):
    import concourse.bass as bass
    import concourse.mybir as mybir
    import concourse.tile as tile
    from concourse import bacc
    from concourse.masks import make_identity

    f32 = mybir.dt.float32
    f32r = mybir.dt.float32r
    bf16 = mybir.dt.bfloat16
    AluOp = mybir.AluOpType
    Act = mybir.ActivationFunctionType

    M3, v3, s3 = host["M3"], host["v3"], host["s3"]

    nc = bacc.Bacc("TRN2", target_bir_lowering=False, debug=False,
                   num_devices=num_devices)

    def din(name, shape, dt):
        return nc.dram_tensor(name, list(shape), dt, kind="ExternalInput").ap()

    ctx_d = din("ctx", [BLOC, C], f32)
    beta_d = din("betav", [BLOC, 1], f32)
    wg_d = din("wg", [128, 29, 4, 128], bf16)
    gbias_d = din("gbias", [29, 128], f32)
    w1t_d = din("w1t", [128, 8, 3], f32)
    convd_d = din("convd", [8, 128, ETOT], bf16)
    xim_d = din("xim", [NBC, 128, 6, FREE], bf16)
    c3wt_d = din("c3wt", [128, 8, C], f32r)
    c3bias_d = din("c3bias", [C, N], f32)
    c4wt_d = din("c4wt", [128, 4, 256], f32r)
    c4b_d = din("c4b", [2, 128], f32)
    clwt_d = din("clwt", [128, 2, PD], f32r)
    clb_d = din("clb", [PD, 1], f32)
    out_d = nc.dram_tensor("out", [BLOC * N, PD], f32, kind="ExternalOutput").ap()
    DEBUG = bool(int(os.environ.get("KERNEL_DEBUG", "0")))
    if DEBUG:
        dbg_nctx = nc.dram_tensor("dbg_nctx", [128, C], f32, kind="ExternalOutput").ap()
        dbg_g = nc.dram_tensor("dbg_g", [128, 2048], f32, kind="ExternalOutput").ap()
        dbg_et = nc.dram_tensor("dbg_et", [128, ETOT], f32, kind="ExternalOutput").ap()
        dbg_y = nc.dram_tensor("dbg_y", [8, 128, BC, N], f32, kind="ExternalOutput").ap()

    with tile.TileContext(nc) as tc:
        import contextlib
        est = contextlib.ExitStack()
        with est:
            wp = est.enter_context(tc.tile_pool(name="wp", bufs=1))
            gout = est.enter_context(tc.tile_pool(name="gout", bufs=1))

            # ---------- persistent small tiles + their DMAs ----------
            # The sim's DMA device drains transfers in enqueue order, and
            # gpsimd issues dma_starts nearly for free -- so ALL prefetches
            # go on the gpsimd queue in explicit priority order:
            # phase-A inputs, gate weights, conv taps (kappa-grouped so E
            # rounds can start before the tail arrives), xim/c3w interleaved.
            ctx_t = wp.tile([128, C], f32, tag="ctx")
            nc.gpsimd.dma_start(ctx_t[:], ctx_d[:])
            beta_t0 = wp.tile([128, 1], f32, tag="beta0")
            nc.gpsimd.dma_start(beta_t0[:], beta_d[:])
            gbias_s = wp.tile([128, 29], f32, tag="gbias")
            nc.gpsimd.dma_start(gbias_s[:], gbias_d.rearrange("c p -> p c"))
            w1t_s = wp.tile([128, 8, 3], f32, tag="w1t")
            nc.gpsimd.dma_start(w1t_s[:], w1t_d[:])
            c3bias_s = wp.tile([128, 4, N], f32, tag="c3bias")
            nc.gpsimd.dma_start(c3bias_s[:], c3bias_d.rearrange("(m p) n -> p m n", p=128))
            c4b_s = wp.tile([128, 2], f32, tag="c4b")
            nc.gpsimd.dma_start(c4b_s[:], c4b_d.rearrange("m p -> p m"))
            clb_s = wp.tile([PD, 1], f32, tag="clb")
            nc.gpsimd.dma_start(clb_s[:], clb_d[:])
            ident = wp.tile([128, 128], f32, tag="ident")
            make_identity(nc, ident[:])

            wg_s = wp.tile([128, 29, 4, 128], bf16, tag="wg")
            nc.gpsimd.dma_start(wg_s[:], wg_d[:])

            # gate/hyper output tiles [feature_part, chunk*16 + b]
            g1b1_s = gout.tile([128, 2048], f32, tag="g1b1")
            g3h3_s = gout.tile([128, 1024], f32, tag="g3h3")
            g4h4_s = gout.tile([128, 512], f32, tag="g4h4")
            gl_s = gout.tile([PD, 128], f32, tag="gl")
            hl_s = gout.tile([PD, 128], f32, tag="hl")

            # ---------- conv tap-weight tiles (pair-grouped layout) ----------
            convd_s = []
            for ci in range(8):
                t = wp.tile([128, ETOT], bf16, tag=f"convd{ci}")
                convd_s.append(t)
            # delta0 block first, then the +-kappa pairs in E-round order
            cgroups = [(0, 1024)] + [(CO_N[k], 2 * WK[k]) for k in range(5)]
            for a_, w_ in cgroups[:4]:
                for ci in range(8):
                    nc.gpsimd.dma_start(convd_s[ci][:, a_:a_ + w_],
                                        convd_d[ci][:, a_:a_ + w_])

            # weights for c3 (first half), then conv tail, then the rest
            c3w_all = wp.tile([128, 8, C], f32r, tag="c3w")
            nc.gpsimd.dma_start(c3w_all[:, :, 0:256], c3wt_d[:, :, 0:256])
            for a_, w_ in cgroups[4:]:
                for ci in range(8):
                    nc.gpsimd.dma_start(convd_s[ci][:, a_:a_ + w_],
                                        convd_d[ci][:, a_:a_ + w_])

            # xim tiles: host-built block-diag im2col, rotated per chunk
            ximp = est.enter_context(tc.tile_pool(name="ximp", bufs=3))
            xim_t = {}

            def fetch_xim(bc):
                t = ximp.tile([128, 6, FREE], bf16, tag="xim")
                nc.gpsimd.dma_start(t[:], xim_d[bc])
                xim_t[bc] = t

            fetch_xim(0)
            nc.gpsimd.dma_start(c3w_all[:, :, 256:C], c3wt_d[:, :, 256:C])
            c4w_all = wp.tile([128, 4, 256], f32r, tag="c4w")
            nc.gpsimd.dma_start(c4w_all[:], c4wt_d[:])
            clw_all = wp.tile([128, 2, PD], f32r, tag="clw")
            nc.gpsimd.dma_start(clw_all[:], clwt_d[:])
            fetch_xim(1)
            c3w_s = [c3w_all[:, k, :] for k in range(8)]
            c4w_s = [c4w_all[:, k, :] for k in range(4)]
            clw_s = [clw_all[:, k, :] for k in range(2)]

            # ---------- phase A: new_ctx ----------
            nctxT = wp.tile([128, C], bf16, tag="nctxT")
            with tc.tile_pool(name="pa", bufs=1) as pap:
                beta_t = beta_t0

                sinb = pap.tile([128, 1], f32, tag="sinb")
                nc.scalar.activation(sinb[:], beta_t[:], Act.Sin)
                cosb = pap.tile([128, 1], f32, tag="cosb")
                nc.vector.tensor_scalar_add(cosb[:], beta_t[:], math.pi / 2)
                nc.scalar.activation(cosb[:], cosb[:], Act.Sin)

                u = pap.tile([128, 3], f32, tag="u")
                for j in range(3):
                    uj = u[:, j:j + 1]
                    nc.vector.tensor_scalar(uj, beta_t[:], float(M3[j, 0]),
                                            float(v3[j]), AluOp.mult, AluOp.add)
                    nc.vector.scalar_tensor_tensor(uj, sinb[:], float(M3[j, 1]),
                                                   uj, AluOp.mult, AluOp.add)
                    nc.vector.scalar_tensor_tensor(uj, cosb[:], float(M3[j, 2]),
                                                   uj, AluOp.mult, AluOp.add)

                with tc.tile_pool(name="ps_a", bufs=1, space="PSUM") as psa:
                    ej = psa.tile([128, C], f32, tag="ej")
                    z = pap.tile([128, C], f32, tag="z")
                    num = pap.tile([128, C], f32, tag="num")
                    tvec = [beta_t, sinb, cosb]
                    for j in range(3):
                        nc.scalar.activation(ej[:], ctx_t[:], Act.Exp,
                                             bias=u[:, j:j + 1], scale=float(s3[j]))
                        if j == 0:
                            nc.vector.tensor_copy(z[:], ej[:])
                            nc.vector.tensor_scalar(num[:], ej[:], tvec[j][:], None,
                                                    AluOp.mult)
                        else:
                            nc.vector.tensor_add(z[:], z[:], ej[:])
                            nc.vector.scalar_tensor_tensor(num[:], ej[:], tvec[j][:],
                                                           num[:], AluOp.mult,
                                                           AluOp.add)
                    nc.scalar.activation(z[:], z[:], Act.Ln)
                    nc.scalar.activation(z[:], z[:], Act.Exp, scale=-1.0)
                    nc.vector.tensor_mul(num[:], num[:], z[:])
                    nctx = ctx_t
                    nc.vector.tensor_add(nctx[:], ctx_t[:], num[:])

                    for kb in range(4):
                        pst = psa.tile([128, 128], f32, tag="tr")
                        nc.tensor.transpose(pst[:],
                                            nctx[:, kb * 128:(kb + 1) * 128],
                                            ident[:])
                        nc.scalar.copy(nctxT[:, kb * 128:(kb + 1) * 128], pst[:])
                    if DEBUG:
                        nc.sync.dma_start(dbg_nctx[:], nctx[:])

            # ---------- phase B: gates (bf16 weights, FD=128) ----------
            with (
                tc.tile_pool(name="ps_b", bufs=8, space="PSUM") as pbp,
            ):
                def gdst(c):
                    if c < 8:
                        return g1b1_s[:, c * 128:(c + 1) * 128], True
                    if c < 16:
                        return g1b1_s[:, 1024 + (c - 8) * 128:1024 + (c - 7) * 128], False
                    if c < 20:
                        return g3h3_s[:, (c - 16) * 128:(c - 15) * 128], True
                    if c < 24:
                        return g3h3_s[:, 512 + (c - 20) * 128:512 + (c - 19) * 128], False
                    if c < 26:
                        return g4h4_s[:, (c - 24) * 128:(c - 23) * 128], True
                    return g4h4_s[:, 256 + (c - 26) * 128:256 + (c - 25) * 128], False

                gp_bank = None
                for c in range(29):
                    if c % 4 == 0:
                        gp_bank = pbp.tile([128, 4, 128], f32, tag="gps")
                    gw_t = wg_s[:, c]
                    for k in range(4):
                        rhs = nctxT[:, k * 128:(k + 1) * 128]
                        if c == 28:
                            nc.tensor.matmul(gp_bank[0:2, 0, :],
                                             gw_t[:, k, 0:2], rhs,
                                             start=(k == 0), stop=False)
                            nc.tensor.matmul(gp_bank[0:2, 1, :],
                                             gw_t[:, k, 2:4], rhs,
                                             start=False, stop=(k == 3))
                        else:
                            nc.tensor.matmul(gp_bank[:, c % 4, :],
                                             gw_t[:, k, :], rhs,
                                             start=(k == 0), stop=(k == 3))
                    if c == 28:
                        nc.scalar.activation(gl_s[:], gp_bank[0:2, 0, :],
                                             Act.Sigmoid,
                                             bias=gbias_s[0:2, 28:29])
                        nc.scalar.copy(hl_s[:], gp_bank[0:2, 1, :])
                    else:
                        dst, is_g = gdst(c)
                        if is_g:
                            nc.scalar.activation(dst, gp_bank[:, c % 4, :],
                                                 Act.Sigmoid,
                                                 bias=gbias_s[:, c:c + 1])
                        else:
                            nc.scalar.copy(dst, gp_bank[:, c % 4, :])

            if DEBUG:
                nc.sync.dma_start(dbg_g[:], g1b1_s[:])

            # ---------- phase C ----------
            with (
                tc.tile_pool(name="g1w1p", bufs=2) as g1w1p,
                tc.tile_pool(name="etp", bufs=2) as etp,
                tc.tile_pool(name="yp", bufs=8) as yp,
                tc.tile_pool(name="t3p", bufs=4) as t3p,
                tc.tile_pool(name="obp", bufs=3) as obp,
                tc.tile_pool(name="ps_e", bufs=2, space="PSUM") as ps_e,
                tc.tile_pool(name="ps_cv", bufs=2, space="PSUM") as ps_cv,
                tc.tile_pool(name="ps_c3", bufs=2, space="PSUM") as ps_c3,
                tc.tile_pool(name="ps_ms", bufs=2, space="PSUM") as ps_ms,
            ):
                def bcast(ap_2d, np_=N):
                    return ap_2d.unsqueeze(2).broadcast_to(
                        [ap_2d.shape[0], BC, np_])

                def build_g1w1(t):
                    """G1W1ext for pair t: [ci_part, fc, (half, j', b^)] bf16.
                    Runs on GpSimd (Pool) — the otherwise-idle engine."""
                    g = g1w1p.tile([128, 8, 2, 4, 16], bf16, tag="g1w1")
                    for fc in range(8):
                        g1s = g1b1_s[:, fc * 128 + t * 32:fc * 128 + t * 32 + 32]
                        g1v = g1s.rearrange("p (h b) -> p h b", h=2)
                        b1s = g1b1_s[:, 1024 + fc * 128 + t * 32:1024 + fc * 128 + t * 32 + 32]
                        b1v = b1s.rearrange("p (h b) -> p h b", h=2)
                        w1v = w1t_s[:, fc, :]
                        nc.gpsimd.tensor_mul(
                            g[:, fc, :, 0:3, :],
                            w1v.unsqueeze(1).unsqueeze(3).broadcast_to([128, 2, 3, 16]),
                            g1v.unsqueeze(2).broadcast_to([128, 2, 3, 16]))
                        nc.gpsimd.tensor_copy(g[:, fc, :, 3, :], b1v)
                    gs = g1w1p.tile([128, 8, 2, 4, 16], bf16, tag="g1w1s")
                    nc.gpsimd.tensor_copy(gs[:, :, 0], g[:, :, 1])
                    nc.gpsimd.tensor_copy(gs[:, :, 1], g[:, :, 0])
                    return g, gs

                def compute_et(t, g1w1_pair):
                    """E_T tiles for pair t.

                    et0 [128, 1024]: delta=0 rows (chunk A at 0:64, B 64:128).
                    ea/eb [128, PTOT]: per-chunk paired tap rows -- kappa
                    span holds delta=-kappa in one half, +kappa in the
                    other, so one 128-row matmul covers both taps. The
                    +kappa E rounds use a half-swapped G1W1 so every evac
                    is a same-partition copy.
                    """
                    g_n, g_s = g1w1_pair
                    et0 = etp.tile([128, 1024], bf16, tag="et0")
                    ea = etp.tile([128, PTOT], bf16, tag="eta")
                    eb = etp.tile([128, PTOT], bf16, tag="etb")
                    rounds = [("z", 0, 0, 512), ("z", 512, 0, 512)]
                    for k in range(5):
                        rounds.append(("n", CO_N[k], int(PAIRO[k]), WK[k]))
                        rounds.append(("p", CO_P[k], int(PAIRO[k]), WK[k]))
                    for ri, (kind, src0, po, w) in enumerate(rounds):
                        pse = ps_e.tile([128, 512], f32, tag="eps")
                        g_use = g_s if kind == "p" else g_n
                        for fc in range(8):
                            nc.tensor.matmul(
                                pse[:, 0:w],
                                g_use[:, fc].rearrange("p h j b -> p (h j b)"),
                                convd_s[fc][:, src0:src0 + w],
                                start=(fc == 0), stop=(fc == 7))
                        ce = nc.scalar if ri % 2 == 0 else nc.vector
                        cp = (ce.copy if ri % 2 == 0
                              else lambda d, s: nc.vector.tensor_copy(d, s))
                        if kind == "z":
                            cp(et0[:, src0:src0 + w], pse[:, 0:w])
                        elif kind == "n":
                            cp(ea[0:64, po:po + w], pse[0:64, 0:w])
                            cp(eb[64:128, po:po + w], pse[64:128, 0:w])
                        else:
                            cp(eb[0:64, po:po + w], pse[0:64, 0:w])
                            cp(ea[64:128, po:po + w], pse[64:128, 0:w])
                    return et0, ea, eb

                def do_chunk(bc, et_t, pre_c3=None):
                    cs = bc * BC
                    h = bc % 2
                    et0, ea, eb = et_t
                    ep = ea if h == 0 else eb
                    xim = xim_t.pop(bc)
                    if bc + 2 < NBC:
                        fetch_xim(bc + 2)

                    # --- apply: trans psum per co-blk; delta0 (64 rows) plus
                    # one paired 128-row matmul per overlapping kappa
                    Y_t = []
                    for co in range(8):
                        psc = ps_cv.tile([128, BC, N], f32, tag="conv")
                        mms = [(None, co * 128, (co + 1) * 128)]
                        for k in range(5):
                            a = max(co * 128, RUN0K[k])
                            b2 = (co + 1) * 128
                            if a < b2:
                                mms.append((k, a, b2))
                        for i, (k, a, b2) in enumerate(mms):
                            if k is None:
                                lhsT = et0[h * 64:h * 64 + 64, a:b2]
                                rhs = xim[h * 64:h * 64 + 64, 0]
                            else:
                                lo = int(PAIRO[k]) + (a - RUN0K[k])
                                lhsT = ep[:, lo:lo + (b2 - a)]
                                rhs = xim[:, k + 1]
                            nc.tensor.matmul(
                                psc[a - co * 128:b2 - co * 128], lhsT, rhs,
                                start=(i == 0), stop=(i == len(mms) - 1))
                        Yc = yp.tile([128, BC, N], f32r, tag="y")
                        nc.scalar.copy(Yc[:], psc[:])
                        if DEBUG and bc == 0:
                            nc.sync.dma_start(dbg_y[co], Yc[:].bitcast(f32))
                        Y_t.append(Yc)

                    # next pair's E_T matmuls slot in here: their PSUM evacs
                    # overlap this chunk's c3/c4/cl PE work.
                    if pre_c3 is not None:
                        pre_c3()

                    # --- c3
                    T3_t = []
                    for m in range(4):
                        ps3 = ps_c3.tile([128, BC, N], f32, tag="c3")
                        for k in range(8):
                            nc.tensor.matmul(
                                ps3[:], c3w_s[k][:, m * 128:(m + 1) * 128],
                                Y_t[k][:], start=(k == 0), stop=(k == 7))
                        T3m = t3p.tile([128, BC, N], f32r, tag="t3")
                        cb = c3bias_s[:, m, :].unsqueeze(1).broadcast_to([128, BC, N])
                        nc.vector.tensor_add(T3m[:], ps3[:], cb)
                        nc.vector.tensor_mul(
                            T3m[:], T3m[:],
                            bcast(g3h3_s[:, m * 128 + cs:m * 128 + cs + BC]))
                        nc.gpsimd.tensor_add(
                            T3m[:], T3m[:],
                            bcast(g3h3_s[:, 512 + m * 128 + cs:512 + m * 128 + cs + BC]))
                        T3_t.append(T3m)

                    # --- c4
                    T4_t = []
                    for m in range(2):
                        ps4 = ps_ms.tile([128, BC, N], f32, tag="ms")
                        for k in range(4):
                            nc.tensor.matmul(
                                ps4[:], c4w_s[k][:, m * 128:(m + 1) * 128],
                                T3_t[k][:], start=(k == 0), stop=(k == 3))
                        T4m = t3p.tile([128, BC, N], f32r, tag="t3")
                        nc.vector.scalar_tensor_tensor(
                            T4m[:], ps4[:], c4b_s[:, m:m + 1],
                            bcast(g4h4_s[:, m * 128 + cs:m * 128 + cs + BC]),
                            AluOp.add, AluOp.mult)
                        nc.gpsimd.tensor_add(
                            T4m[:], T4m[:],
                            bcast(g4h4_s[:, 256 + m * 128 + cs:256 + m * 128 + cs + BC]))
                        T4_t.append(T4m)

                    # --- cl
                    psl_full = ps_ms.tile([128, BC, N], f32, tag="ms")
                    psl = psl_full[0:PD]
                    for k in range(2):
                        nc.tensor.matmul(psl[:], clw_s[k], T4_t[k][:],
                                         start=(k == 0), stop=(k == 1))
                    OF_full = t3p.tile([128, BC, N], f32, tag="t3")
                    OF = OF_full[0:PD]
                    gl = gl_s[:, cs:cs + BC].unsqueeze(2).broadcast_to([PD, BC, N])
                    hl = hl_s[:, cs:cs + BC].unsqueeze(2).broadcast_to([PD, BC, N])
                    nc.vector.scalar_tensor_tensor(OF[:], psl[:], clb_s[:], gl,
                                                   AluOp.add, AluOp.mult)
                    nc.gpsimd.tensor_add(OF[:], OF[:], hl)

                    # --- transpose [2, 384] -> [384, 2] in 128-blocks, DMA out
                    OFf = OF[:].rearrange("p b n -> p (b n)")
                    osb = obp.tile([128, 3, PD], f32, tag="ob")
                    for blk in range(3):
                        ptr_full = ps_ms.tile([128, BC, N], f32, tag="ms")
                        ptr = ptr_full.rearrange("p b n -> p (b n)")[:, 0:PD]
                        nc.tensor.transpose(ptr[:], OFf[:, blk * 128:(blk + 1) * 128],
                                            ident[0:PD, 0:PD])
                        nc.scalar.copy(osb[:, blk, :], ptr[:])
                    row0 = bc * 384
                    oap = out_d[row0:row0 + 384, :].rearrange(
                        "(blk p) c -> p blk c", blk=3, p=128)
                    nc.sync.dma_start(oap, osb[:])

                def _phase_c():
                    g_cur = build_g1w1(0)
                    et_cur = compute_et(0, g_cur)
                    for t in range(NPAIR):
                        do_chunk(2 * t, et_cur)
                        if t + 1 < NPAIR:
                            g_next = build_g1w1(t + 1)
                            nxt = []
                            do_chunk(2 * t + 1, et_cur,
                                     pre_c3=lambda tt=t + 1, g=g_next, nxt=nxt:
                                         nxt.append(compute_et(tt, g)))
                            et_cur = nxt[0]
                        else:
                            do_chunk(2 * t + 1, et_cur)

                LOOPN = int(os.environ.get("KERNEL_LOOP", "1"))
                if LOOPN > 1:
                    with tc.For_i(0, LOOPN, 1):
                        _phase_c()
                else:
                    _phase_c()

    nc.compile()
    return nc


def _build_and_run(host, in_maps, trace):
    from concourse.bass_utils import run_bass_kernel_spmd

    nc = _build(host)
    res = run_bass_kernel_spmd(
        nc, in_maps, core_ids=list(range(NCORES)), trace=trace,
        trace_cores=list(range(NCORES)) if trace else None,
        stitch_traces=bool(trace and NCORES > 1))
    return res


def _host_prep(**inputs):
    x = _f32(inputs["x"])
    beta = _f32(inputs["beta"])
    context = _f32(inputs["context"])
    g = {k: np.asarray(v, dtype=np.float64) for k, v in inputs.items()
         if k not in ("x", "beta", "context")}

    # --- algebraic folds (host, tiny) ---
    embW = g["emb_w"][:, :, 0]            # [64, 3]
    dembW = g["demb_w"][:, :, 0]          # [3, 64]
    M3 = dembW @ embW                     # [3, 3]
    v3 = dembW @ g["emb_b"] + g["demb_b"]
    s3 = M3.sum(axis=1)

    pe = _pe_table().astype(np.float64)   # [N, F]

    c1aug = np.empty((3, F), np.float64)
    c1aug[0:2] = g["c1_w"].T
    c1aug[2] = g["c1_b"]

    # gate weights [C, 29*128]
    wg = np.zeros((C, 29 * 128), np.float32)
    wg[:, 0:1024] = g["c1_gw"].T
    wg[:, 1024:2048] = g["c1_hw"].T
    wg[:, 2048:2560] = g["c3_gw"].T
    wg[:, 2560:3072] = g["c3_hw"].T
    wg[:, 3072:3328] = g["c4_gw"].T
    wg[:, 3328:3584] = g["c4_hw"].T
    wg[:, 3584:3586] = g["cl_gw"].T
    wg[:, 3586:3588] = g["cl_hw"].T
    gbias = np.zeros(29 * 128, np.float32)
    gbias[0:1024] = g["c1_gb"]
    gbias[2048:2560] = g["c3_gb"]
    gbias[3072:3328] = g["c4_gb"]
    gbias[3584:3586] = g["cl_gb"]
    gbias = gbias.reshape(29, 128)
    # [p, c, k, o] = wg[k*128+p, c*128+o]: one prefetched DMA, sliced per c
    wg = np.ascontiguousarray(
        wg.reshape(4, 128, 29, 128).transpose(1, 2, 0, 3))

    # conv weights -> [11, ci, co] tap-major with zero padding
    convt = np.zeros((11, F, F), np.float64)
    convt[5, :, 0:512] = g["conv1_w"][:, :, 0].T
    for t in range(3):
        convt[t + 4, :, 512:768] = g["conv2_w"][:, :, t].T
    for t in range(5):
        convt[t + 3, :, 768:832] = g["conv3_w"][:, :, t].T
    for t in range(7):
        convt[t + 2, :, 832:896] = g["conv4_w"][:, :, t].T
    for t in range(9):
        convt[t + 1, :, 896:960] = g["conv5_w"][:, :, t].T
    for t in range(11):
        convt[t, :, 960:1024] = g["conv6_w"][:, :, t].T

    # pair-grouped conv layout: [delta0 | -1,+1 | -2,+2 | ... | -5,+5]
    convd = np.empty((8, 128, ETOT), np.float32)
    convd[:, :, 0:1024] = convt[5].reshape(8, 128, 1024)
    for k in range(5):
        wk, r0 = WK[k], RUN0K[k]
        convd[:, :, CO_N[k]:CO_N[k] + wk] = \
            convt[5 - (k + 1), :, r0:1024].reshape(8, 128, wk)
        convd[:, :, CO_P[k]:CO_P[k] + wk] = \
            convt[5 + (k + 1), :, r0:1024].reshape(8, 128, wk)

    # positional encoding pushed through the convs (host, exact):
    peT = pe.T                             # [F, N] float64
    pe_conv = np.zeros((F, N), np.float64)
    for d in range(-5, 6):
        a, b2 = max(0, -d), N - max(0, d)
        pe_conv[:, a:b2] += convt[d + 5].T @ peT[:, a + d:b2 + d]
    conv_bias = np.concatenate([g["conv1_b"], g["conv2_b"], g["conv3_b"],
                                g["conv4_b"], g["conv5_b"], g["conv6_b"]])
    c3bias = (g["c3_w"] @ (pe_conv + conv_bias[:, None])
              + g["c3_b"][:, None]).astype(np.float32)   # [C, N]

    # W1T[p, fc, j] = c1aug[j, fc*128+p]
    w1t = _f32(np.ascontiguousarray(
        c1aug.reshape(3, 8, 128).transpose(2, 1, 0)))

    c3wt = _f32(g["c3_w"].T.reshape(8, 128, C).transpose(1, 0, 2))
    c4wt = _f32(g["c4_w"].T.reshape(4, 128, 256).transpose(1, 0, 2))
    c4b = _f32(g["c4_b"].reshape(2, 128))
    clwt = _f32(g["cl_w"].T.reshape(2, 128, PD).transpose(1, 0, 2))
    clb = _f32(g["cl_b"].reshape(PD, 1))

    host = dict(M3=M3, v3=v3, s3=s3)

    shared = dict(wg=_bf16(wg), gbias=gbias, w1t=w1t, convd=_bf16(convd),
                  c3wt=c3wt, c3bias=c3bias, c4wt=c4wt, c4b=c4b,
                  clwt=clwt, clb=clb)

    # xim: block-diag im2col of x (+ bias-mask rows).
    # slot 0 = delta0 (chunk rows duplicated in both halves); slot kappa
    # holds -kappa/+kappa in opposite halves, swapped for odd chunks to
    # match the E-tile pairing.
    xaug = np.empty((3, B, N), np.float32)
    xaug[0:2] = x.transpose(2, 0, 1)
    xaug[2] = 1.0
    in_maps = []
    for k in range(NCORES):
        sl = slice(k * BLOC, (k + 1) * BLOC)
        xim = np.zeros((NBC, 128, 6, FREE), np.float32)
        for bc in range(NBC):
            par = bc % 2
            for sidx in range(6):
                for half in (0, 1):
                    if sidx == 0:
                        dlt = 0
                    else:
                        sgn = -1 if (half == par) else 1
                        dlt = sgn * sidx
                    n0, n1 = max(0, -dlt), min(N, N - dlt)
                    for bh in range(BC):
                        gb = k * BLOC + bc * BC + bh
                        col0 = bh * N
                        for jp in range(3):
                            xim[bc, half * 64 + jp * 16 + bh, sidx,
                                col0 + n0:col0 + n1] = \
                                xaug[jp, gb, n0 + dlt:n1 + dlt]
                        xim[bc, half * 64 + 48 + bh, sidx,
                            col0 + n0:col0 + n1] = 1.0
        m = dict(shared)
        m["ctx"] = np.ascontiguousarray(context[sl])
        m["betav"] = np.ascontiguousarray(beta[sl].reshape(BLOC, 1))
        m["xim"] = _bf16(xim)
        in_maps.append(m)

    return host, in_maps


_LAST_HOST = None


def kernel(**inputs):
    global LAST_RESULTS, _LAST_HOST
    host, in_maps = _host_prep(**inputs)
    _LAST_HOST = host
    trace = bool(int(os.environ.get("KERNEL_TRACE", "0")))
    res = _build_and_run(host, in_maps, trace)
    LAST_RESULTS = res
    out = np.concatenate(
        [res.results[k]["out"].reshape(BLOC, N, PD) for k in range(NCORES)],
        axis=0)
    return out


# revision 17
# speedup vs baseline: 1.2190x; 1.2190x over previous
"""Trainium2 Bass kernel for nn_CNNConcatLinear (B=1024, N=24, PD=2, C=512).

Strategy: pure data-parallel over batch (128 per core x 8 cores).

Algebraic restructure of the conv stack: the conv input X is rank-4 per
batch sample (2 x-dims + c1-bias, all gated, plus the ungated hyper-bias),
so instead of contracting 1024 channels x 384 positions per chunk, we:

  phase A: new_ctx via the exact 3x3 softmax fold (as before).
  phase B: all CSL gates/hyper-biases as [feature, batch] matmuls (bf16).
  phase E (per 16x2-batch pair): effective per-batch tap weights
      E_T[(half,j',b^), co] = sum_ci G1W1ext[ci,(half,j',b^)] * W_delta[ci,co]
    where G1W1ext packs c1_w*g1 (j'<3) and the hyper-bias b1 (j'=3),
    and W_delta are the conv taps in a suffix-run co layout (runs are
    contiguous because tap sets nest: co>=RUN0[delta]).
  apply (per 16-batch chunk): trans[co, b, n] = sum_(delta,j',b^)
      E_T[...] * xim[...] against a host-built block-diagonal im2col of x
    (plus edge-mask rows for the bias term); positional encoding and conv
    biases are host-folded into the c3 bias table.
  then c3/c4/cl CSL layers exactly as before (f32r matmuls, FD=384).

bf16 is used for all matmuls whose free dim is < 256 (f32r costs 4
cycles/row there) and for the E/apply path; c3/c4/cl stay f32r.
"""

import math
import os

import numpy as np
import ml_dtypes

B, N, PD, C = 1024, 24, 2, 512
F = 2 * C
NCORES = 8
BLOC = B // NCORES          # 128 batch per core
BC = 16                     # batch chunk
NBC = BLOC // BC            # 8 chunks
NPAIR = NBC // 2            # 4 chunk-pairs
FREE = BC * N               # 384

# Tap structure: tap sets nest (each conv's taps are a prefix of
# [0, +-1, ..., +-5]); co-runs for |delta|=kappa start at RUN0K[kappa-1].
RUN0K = [512, 768, 832, 896, 960]           # run start for kappa=1..5
WK = [1024 - r for r in RUN0K]              # run width: 512,256,192,128,64
PAIRO = np.concatenate([[0], np.cumsum(WK)]).astype(int)  # offsets in pair tiles
PTOT = int(PAIRO[-1])                       # 1152
# convd column layout: [delta0 (1024) | -1,+1 | -2,+2 | ... | -5,+5]
CO_N = [1024 + 2 * int(PAIRO[k]) for k in range(5)]      # neg slice start
CO_P = [CO_N[k] + WK[k] for k in range(5)]               # pos slice start
ETOT = 1024 + 2 * PTOT                      # 3328

LAST_RESULTS = None         # BassKernelResults from the most recent run


def _pe_table():
    pos = np.arange(N, dtype=np.float32)[:, None]
    div = np.exp(np.arange(0, F, 2, dtype=np.float32) * (-np.log(10000.0) / F))
    pe = np.zeros((N, F), dtype=np.float32)
    pe[:, 0::2] = np.sin(pos * div)
    pe[:, 1::2] = np.cos(pos * div)
    return pe


def _f32(a):
    return np.ascontiguousarray(np.asarray(a, dtype=np.float32))


def _bf16(a):
    return np.ascontiguousarray(np.asarray(a, dtype=ml_dtypes.bfloat16))


def _build(host, num_devices=NCORES):
    import concourse.bass as bass
    import concourse.mybir as mybir
    import concourse.tile as tile
    from concourse import bacc
    from concourse.masks import make_identity

    f32 = mybir.dt.float32
    f32r = mybir.dt.float32r
    bf16 = mybir.dt.bfloat16
    AluOp = mybir.AluOpType
    Act = mybir.ActivationFunctionType

    M3, v3, s3 = host["M3"], host["v3"], host["s3"]

    nc = bacc.Bacc("TRN2", target_bir_lowering=False, debug=False,
                   num_devices=num_devices)

    def din(name, shape, dt):
        return nc.dram_tensor(name, list(shape), dt, kind="ExternalInput").ap()

    ctx_d = din("ctx", [BLOC, C], f32)
    beta_d = din("betav", [BLOC, 1], f32)
    wg_d = din("wg", [128, 29, 4, 128], bf16)
    gbias_d = din("gbias", [29, 128], f32)
    w1t_d = din("w1t", [128, 8, 3], f32)
    convd_d = din("convd", [8, 128, ETOT], bf16)
    xim_d = din("xim", [NBC, 128, 6, FREE], bf16)
    c3wt_d = din("c3wt", [128, 8, C], f32r)
    c3bias_d = din("c3bias", [C, N], f32)
    c4wt_d = din("c4wt", [128, 4, 256], f32r)
    c4b_d = din("c4b", [2, 128], f32)
    clwt_d = din("clwt", [128, 2, PD], f32r)
    clb_d = din("clb", [PD, 1], f32)
    out_d = nc.dram_tensor("out", [BLOC * N, PD], f32, kind="ExternalOutput").ap()
    DEBUG = bool(int(os.environ.get("KERNEL_DEBUG", "0")))
    if DEBUG:
        dbg_nctx = nc.dram_tensor("dbg_nctx", [128, C], f32, kind="ExternalOutput").ap()
        dbg_g = nc.dram_tensor("dbg_g", [128, 2048], f32, kind="ExternalOutput").ap()
        dbg_et = nc.dram_tensor("dbg_et", [128, ETOT], f32, kind="ExternalOutput").ap()
        dbg_y = nc.dram_tensor("dbg_y", [8, 128, BC, N], f32, kind="ExternalOutput").ap()

    with tile.TileContext(nc) as tc:
        import contextlib
        est = contextlib.ExitStack()
        with est:
            wp = est.enter_context(tc.tile_pool(name="wp", bufs=1))
            gout = est.enter_context(tc.tile_pool(name="gout", bufs=1))

            # ---------- persistent small tiles + their DMAs ----------
            # The sim's DMA device drains transfers in enqueue order, and
            # gpsimd issues dma_starts nearly for free -- so ALL prefetches
            # go on the gpsimd queue in explicit priority order:
            # phase-A inputs, gate weights, conv taps (kappa-grouped so E
            # rounds can start before the tail arrives), xim/c3w interleaved.
            ctx_t = wp.tile([128, C], f32, tag="ctx")
            nc.sync.dma_start(ctx_t[:], ctx_d[:])
            beta_t0 = wp.tile([128, 1], f32, tag="beta0")
            nc.sync.dma_start(beta_t0[:], beta_d[:])
            gbias_s = wp.tile([128, 29], f32, tag="gbias")
            nc.sync.dma_start(gbias_s[:], gbias_d.rearrange("c p -> p c"))
            w1t_s = wp.tile([128, 8, 3], f32, tag="w1t")
            nc.sync.dma_start(w1t_s[:], w1t_d[:])
            c3bias_s = wp.tile([128, 4, N], f32, tag="c3bias")
            nc.sync.dma_start(c3bias_s[:], c3bias_d.rearrange("(m p) n -> p m n", p=128))
            c4b_s = wp.tile([128, 2], f32, tag="c4b")
            nc.sync.dma_start(c4b_s[:], c4b_d.rearrange("m p -> p m"))
            clb_s = wp.tile([PD, 1], f32, tag="clb")
            nc.sync.dma_start(clb_s[:], clb_d[:])
            ident = wp.tile([128, 128], f32, tag="ident")
            make_identity(nc, ident[:])

            wg_s = wp.tile([128, 29, 4, 128], bf16, tag="wg")
            nc.sync.dma_start(wg_s[:], wg_d[:])

            # gate/hyper output tiles [feature_part, chunk*16 + b]
            g1b1_s = gout.tile([128, 2048], f32, tag="g1b1")
            g3h3_s = gout.tile([128, 1024], f32, tag="g3h3")
            g4h4_s = gout.tile([128, 512], f32, tag="g4h4")
            gl_s = gout.tile([PD, 128], f32, tag="gl")
            hl_s = gout.tile([PD, 128], f32, tag="hl")

            # ---------- conv tap-weight tiles (pair-grouped layout) ----------
            convd_s = []
            for ci in range(8):
                t = wp.tile([128, ETOT], bf16, tag=f"convd{ci}")
                convd_s.append(t)
            # delta0 block first, then the +-kappa pairs in E-round order
            cgroups = [(0, 1024)] + [(CO_N[k], 2 * WK[k]) for k in range(5)]
            for a_, w_ in cgroups:
                for ci in range(8):
                    nc.sync.dma_start(convd_s[ci][:, a_:a_ + w_],
                                      convd_d[ci][:, a_:a_ + w_])

            # xim tiles: host-built block-diag im2col, rotated per chunk
            ximp = est.enter_context(tc.tile_pool(name="ximp", bufs=3))
            xim_t = {}

            def fetch_xim(bc):
                t = ximp.tile([128, 6, FREE], bf16, tag="xim")
                nc.sync.dma_start(t[:], xim_d[bc])
                xim_t[bc] = t

            fetch_xim(0)
            c3w_all = wp.tile([128, 8, C], f32r, tag="c3w")
            nc.sync.dma_start(c3w_all[:, :, 0:256], c3wt_d[:, :, 0:256])
            nc.sync.dma_start(c3w_all[:, :, 256:C], c3wt_d[:, :, 256:C])
            fetch_xim(1)
            c4w_all = wp.tile([128, 4, 256], f32r, tag="c4w")
            nc.sync.dma_start(c4w_all[:], c4wt_d[:])
            clw_all = wp.tile([128, 2, PD], f32r, tag="clw")
            nc.sync.dma_start(clw_all[:], clwt_d[:])
            c3w_s = [c3w_all[:, k, :] for k in range(8)]
            c4w_s = [c4w_all[:, k, :] for k in range(4)]
            clw_s = [clw_all[:, k, :] for k in range(2)]

            # ---------- phase A: new_ctx ----------
            nctxT = wp.tile([128, C], bf16, tag="nctxT")
            with tc.tile_pool(name="pa", bufs=1) as pap:
                beta_t = beta_t0

                sinb = pap.tile([128, 1], f32, tag="sinb")
                nc.scalar.activation(sinb[:], beta_t[:], Act.Sin)
                cosb = pap.tile([128, 1], f32, tag="cosb")
                nc.vector.tensor_scalar_add(cosb[:], beta_t[:], math.pi / 2)
                nc.scalar.activation(cosb[:], cosb[:], Act.Sin)

                u = pap.tile([128, 3], f32, tag="u")
                for j in range(3):
                    uj = u[:, j:j + 1]
                    nc.vector.tensor_scalar(uj, beta_t[:], float(M3[j, 0]),
                                            float(v3[j]), AluOp.mult, AluOp.add)
                    nc.vector.scalar_tensor_tensor(uj, sinb[:], float(M3[j, 1]),
                                                   uj, AluOp.mult, AluOp.add)
                    nc.vector.scalar_tensor_tensor(uj, cosb[:], float(M3[j, 2]),
                                                   uj, AluOp.mult, AluOp.add)

                with tc.tile_pool(name="ps_a", bufs=1, space="PSUM") as psa:
                    ej = psa.tile([128, C], f32, tag="ej")
                    z = pap.tile([128, C], f32, tag="z")
                    num = pap.tile([128, C], f32, tag="num")
                    tvec = [beta_t, sinb, cosb]
                    for j in range(3):
                        nc.scalar.activation(ej[:], ctx_t[:], Act.Exp,
                                             bias=u[:, j:j + 1], scale=float(s3[j]))
                        if j == 0:
                            nc.vector.tensor_copy(z[:], ej[:])
                            nc.vector.tensor_scalar(num[:], ej[:], tvec[j][:], None,
                                                    AluOp.mult)
                        else:
                            nc.vector.tensor_add(z[:], z[:], ej[:])
                            nc.vector.scalar_tensor_tensor(num[:], ej[:], tvec[j][:],
                                                           num[:], AluOp.mult,
                                                           AluOp.add)
                    nc.scalar.activation(z[:], z[:], Act.Ln)
                    nc.scalar.activation(z[:], z[:], Act.Exp, scale=-1.0)
                    nc.vector.tensor_mul(num[:], num[:], z[:])
                    nctx = ctx_t
                    nc.vector.tensor_add(nctx[:], ctx_t[:], num[:])

                    for kb in range(4):
                        pst = psa.tile([128, 128], f32, tag="tr")
                        nc.tensor.transpose(pst[:],
                                            nctx[:, kb * 128:(kb + 1) * 128],
                                            ident[:])
                        nc.scalar.copy(nctxT[:, kb * 128:(kb + 1) * 128], pst[:])
                    if DEBUG:
                        nc.sync.dma_start(dbg_nctx[:], nctx[:])

            # ---------- phase B: gates (bf16 weights, FD=128) ----------
            with (
                tc.tile_pool(name="ps_b", bufs=8, space="PSUM") as pbp,
            ):
                def gdst(c):
                    if c < 8:
                        return g1b1_s[:, c * 128:(c + 1) * 128], True
                    if c < 16:
                        return g1b1_s[:, 1024 + (c - 8) * 128:1024 + (c - 7) * 128], False
                    if c < 20:
                        return g3h3_s[:, (c - 16) * 128:(c - 15) * 128], True
                    if c < 24:
                        return g3h3_s[:, 512 + (c - 20) * 128:512 + (c - 19) * 128], False
                    if c < 26:
                        return g4h4_s[:, (c - 24) * 128:(c - 23) * 128], True
                    return g4h4_s[:, 256 + (c - 26) * 128:256 + (c - 25) * 128], False

                gp_bank = None
                for c in range(29):
                    if c % 4 == 0:
                        gp_bank = pbp.tile([128, 4, 128], f32, tag="gps")
                    gw_t = wg_s[:, c]
                    for k in range(4):
                        rhs = nctxT[:, k * 128:(k + 1) * 128]
                        if c == 28:
                            nc.tensor.matmul(gp_bank[0:2, 0, :],
                                             gw_t[:, k, 0:2], rhs,
                                             start=(k == 0), stop=False)
                            nc.tensor.matmul(gp_bank[0:2, 1, :],
                                             gw_t[:, k, 2:4], rhs,
                                             start=False, stop=(k == 3))
                        else:
                            nc.tensor.matmul(gp_bank[:, c % 4, :],
                                             gw_t[:, k, :], rhs,
                                             start=(k == 0), stop=(k == 3))
                    if c == 28:
                        nc.scalar.activation(gl_s[:], gp_bank[0:2, 0, :],
                                             Act.Sigmoid,
                                             bias=gbias_s[0:2, 28:29])
                        nc.scalar.copy(hl_s[:], gp_bank[0:2, 1, :])
                    else:
                        dst, is_g = gdst(c)
                        if is_g:
                            nc.scalar.activation(dst, gp_bank[:, c % 4, :],
                                                 Act.Sigmoid,
                                                 bias=gbias_s[:, c:c + 1])
                        else:
                            nc.scalar.copy(dst, gp_bank[:, c % 4, :])

            if DEBUG:
                nc.sync.dma_start(dbg_g[:], g1b1_s[:])

            # ---------- phase C ----------
            with (
                tc.tile_pool(name="g1w1p", bufs=2) as g1w1p,
                tc.tile_pool(name="etp", bufs=2) as etp,
                tc.tile_pool(name="yp", bufs=8) as yp,
                tc.tile_pool(name="t3p", bufs=4) as t3p,
                tc.tile_pool(name="obp", bufs=3) as obp,
                tc.tile_pool(name="ps_e", bufs=2, space="PSUM") as ps_e,
                tc.tile_pool(name="ps_cv", bufs=2, space="PSUM") as ps_cv,
                tc.tile_pool(name="ps_c3", bufs=2, space="PSUM") as ps_c3,
                tc.tile_pool(name="ps_ms", bufs=2, space="PSUM") as ps_ms,
            ):
                def bcast(ap_2d, np_=N):
                    return ap_2d.unsqueeze(2).broadcast_to(
                        [ap_2d.shape[0], BC, np_])

                def build_g1w1(t):
                    """G1W1ext for pair t: [ci_part, fc, (half, j', b^)] bf16.
                    Runs on GpSimd (Pool) — the otherwise-idle engine."""
                    g = g1w1p.tile([128, 8, 2, 4, 16], bf16, tag="g1w1")
                    for fc in range(8):
                        g1s = g1b1_s[:, fc * 128 + t * 32:fc * 128 + t * 32 + 32]
                        g1v = g1s.rearrange("p (h b) -> p h b", h=2)
                        b1s = g1b1_s[:, 1024 + fc * 128 + t * 32:1024 + fc * 128 + t * 32 + 32]
                        b1v = b1s.rearrange("p (h b) -> p h b", h=2)
                        w1v = w1t_s[:, fc, :]
                        nc.gpsimd.tensor_mul(
                            g[:, fc, :, 0:3, :],
                            w1v.unsqueeze(1).unsqueeze(3).broadcast_to([128, 2, 3, 16]),
                            g1v.unsqueeze(2).broadcast_to([128, 2, 3, 16]))
                        nc.gpsimd.tensor_copy(g[:, fc, :, 3, :], b1v)
                    gs = g1w1p.tile([128, 8, 2, 4, 16], bf16, tag="g1w1s")
                    nc.gpsimd.tensor_copy(gs[:, :, 0], g[:, :, 1])
                    nc.gpsimd.tensor_copy(gs[:, :, 1], g[:, :, 0])
                    return g, gs

                def compute_et(t, g1w1_pair):
                    """E_T tiles for pair t.

                    et0 [128, 1024]: delta=0 rows (chunk A at 0:64, B 64:128).
                    ea/eb [128, PTOT]: per-chunk paired tap rows -- kappa
                    span holds delta=-kappa in one half, +kappa in the
                    other, so one 128-row matmul covers both taps. The
                    +kappa E rounds use a half-swapped G1W1 so every evac
                    is a same-partition copy.
                    """
                    g_n, g_s = g1w1_pair
                    et0 = etp.tile([128, 1024], bf16, tag="et0")
                    ea = etp.tile([128, PTOT], bf16, tag="eta")
                    eb = etp.tile([128, PTOT], bf16, tag="etb")
                    rounds = [("z", 0, 0, 512), ("z", 512, 0, 512)]
                    for k in range(5):
                        rounds.append(("n", CO_N[k], int(PAIRO[k]), WK[k]))
                        rounds.append(("p", CO_P[k], int(PAIRO[k]), WK[k]))
                    for ri, (kind, src0, po, w) in enumerate(rounds):
                        pse = ps_e.tile([128, 512], f32, tag="eps")
                        g_use = g_s if kind == "p" else g_n
                        for fc in range(8):
                            nc.tensor.matmul(
                                pse[:, 0:w],
                                g_use[:, fc].rearrange("p h j b -> p (h j b)"),
                                convd_s[fc][:, src0:src0 + w],
                                start=(fc == 0), stop=(fc == 7))
                        ce = nc.scalar if ri % 2 == 0 else nc.vector
                        cp = (ce.copy if ri % 2 == 0
                              else lambda d, s: nc.vector.tensor_copy(d, s))
                        if kind == "z":
                            cp(et0[:, src0:src0 + w], pse[:, 0:w])
                        elif kind == "n":
                            cp(ea[0:64, po:po + w], pse[0:64, 0:w])
                            cp(eb[64:128, po:po + w], pse[64:128, 0:w])
                        else:
                            cp(eb[0:64, po:po + w], pse[0:64, 0:w])
                            cp(ea[64:128, po:po + w], pse[64:128, 0:w])
                    return et0, ea, eb

                def do_chunk(bc, et_t, pre_c3=None):
                    cs = bc * BC
                    h = bc % 2
                    et0, ea, eb = et_t
                    ep = ea if h == 0 else eb
                    xim = xim_t.pop(bc)
                    if bc + 2 < NBC:
                        fetch_xim(bc + 2)

                    # --- apply: trans psum per co-blk; delta0 (64 rows) plus
                    # one paired 128-row matmul per overlapping kappa
                    Y_t = []
                    for co in range(8):
                        psc = ps_cv.tile([128, BC, N], f32, tag="conv")
                        mms = [(None, co * 128, (co + 1) * 128)]
                        for k in range(5):
                            a = max(co * 128, RUN0K[k])
                            b2 = (co + 1) * 128
                            if a < b2:
                                mms.append((k, a, b2))
                        for i, (k, a, b2) in enumerate(mms):
                            if k is None:
                                lhsT = et0[h * 64:h * 64 + 64, a:b2]
                                rhs = xim[h * 64:h * 64 + 64, 0]
                            else:
                                lo = int(PAIRO[k]) + (a - RUN0K[k])
                                lhsT = ep[:, lo:lo + (b2 - a)]
                                rhs = xim[:, k + 1]
                            nc.tensor.matmul(
                                psc[a - co * 128:b2 - co * 128], lhsT, rhs,
                                start=(i == 0), stop=(i == len(mms) - 1))
                        Yc = yp.tile([128, BC, N], f32r, tag="y")
                        nc.scalar.copy(Yc[:], psc[:])
                        if DEBUG and bc == 0:
                            nc.sync.dma_start(dbg_y[co], Yc[:].bitcast(f32))
                        Y_t.append(Yc)

                    # next pair's E_T matmuls slot in here: their PSUM evacs
                    # overlap this chunk's c3/c4/cl PE work.
                    if pre_c3 is not None:
                        pre_c3()

                    # --- c3
                    T3_t = []
                    for m in range(4):
                        ps3 = ps_c3.tile([128, BC, N], f32, tag="c3")
                        for k in range(8):
                            nc.tensor.matmul(
                                ps3[:], c3w_s[k][:, m * 128:(m + 1) * 128],
                                Y_t[k][:], start=(k == 0), stop=(k == 7))
                        T3m = t3p.tile([128, BC, N], f32r, tag="t3")
                        cb = c3bias_s[:, m, :].unsqueeze(1).broadcast_to([128, BC, N])
                        nc.vector.tensor_add(T3m[:], ps3[:], cb)
                        nc.vector.tensor_mul(
                            T3m[:], T3m[:],
                            bcast(g3h3_s[:, m * 128 + cs:m * 128 + cs + BC]))
                        nc.gpsimd.tensor_add(
                            T3m[:], T3m[:],
                            bcast(g3h3_s[:, 512 + m * 128 + cs:512 + m * 128 + cs + BC]))
                        T3_t.append(T3m)

                    # --- c4
                    T4_t = []
                    for m in range(2):
                        ps4 = ps_ms.tile([128, BC, N], f32, tag="ms")
                        for k in range(4):
                            nc.tensor.matmul(
                                ps4[:], c4w_s[k][:, m * 128:(m + 1) * 128],
                                T3_t[k][:], start=(k == 0), stop=(k == 3))
                        T4m = t3p.tile([128, BC, N], f32r, tag="t3")
                        nc.vector.scalar_tensor_tensor(
                            T4m[:], ps4[:], c4b_s[:, m:m + 1],
                            bcast(g4h4_s[:, m * 128 + cs:m * 128 + cs + BC]),
                            AluOp.add, AluOp.mult)
                        nc.gpsimd.tensor_add(
                            T4m[:], T4m[:],
                            bcast(g4h4_s[:, 256 + m * 128 + cs:256 + m * 128 + cs + BC]))
                        T4_t.append(T4m)

                    # --- cl
                    psl_full = ps_ms.tile([128, BC, N], f32, tag="ms")
                    psl = psl_full[0:PD]
                    for k in range(2):
                        nc.tensor.matmul(psl[:], clw_s[k], T4_t[k][:],
                                         start=(k == 0), stop=(k == 1))
                    OF_full = t3p.tile([128, BC, N], f32, tag="t3")
                    OF = OF_full[0:PD]
                    gl = gl_s[:, cs:cs + BC].unsqueeze(2).broadcast_to([PD, BC, N])
                    hl = hl_s[:, cs:cs + BC].unsqueeze(2).broadcast_to([PD, BC, N])
                    nc.vector.scalar_tensor_tensor(OF[:], psl[:], clb_s[:], gl,
                                                   AluOp.add, AluOp.mult)
                    nc.gpsimd.tensor_add(OF[:], OF[:], hl)

                    # --- transpose [2, 384] -> [384, 2] in 128-blocks, DMA out
                    OFf = OF[:].rearrange("p b n -> p (b n)")
                    osb = obp.tile([128, 3, PD], f32, tag="ob")
                    for blk in range(3):
                        ptr_full = ps_ms.tile([128, BC, N], f32, tag="ms")
                        ptr = ptr_full.rearrange("p b n -> p (b n)")[:, 0:PD]
                        nc.tensor.transpose(ptr[:], OFf[:, blk * 128:(blk + 1) * 128],
                                            ident[0:PD, 0:PD])
                        nc.scalar.copy(osb[:, blk, :], ptr[:])
                    row0 = bc * 384
                    oap = out_d[row0:row0 + 384, :].rearrange(
                        "(blk p) c -> p blk c", blk=3, p=128)
                    nc.sync.dma_start(oap, osb[:])

                def _phase_c():
                    g_cur = build_g1w1(0)
                    et_cur = compute_et(0, g_cur)
                    for t in range(NPAIR):
                        do_chunk(2 * t, et_cur)
                        if t + 1 < NPAIR:
                            g_next = build_g1w1(t + 1)
                            nxt = []
                            do_chunk(2 * t + 1, et_cur,
                                     pre_c3=lambda tt=t + 1, g=g_next, nxt=nxt:
                                         nxt.append(compute_et(tt, g)))
                            et_cur = nxt[0]
                        else:
                            do_chunk(2 * t + 1, et_cur)

                LOOPN = int(os.environ.get("KERNEL_LOOP", "1"))
                if LOOPN > 1:
                    with tc.For_i(0, LOOPN, 1):
                        _phase_c()
                else:
                    _phase_c()

    nc.compile()
    return nc


def _build_and_run(host, in_maps, trace):
    from concourse.bass_utils import run_bass_kernel_spmd

    nc = _build(host)
    res = run_bass_kernel_spmd(
        nc, in_maps, core_ids=list(range(NCORES)), trace=trace,
        trace_cores=list(range(NCORES)) if trace else None,
        stitch_traces=bool(trace and NCORES > 1))
    return res


def _host_prep(**inputs):
    x = _f32(inputs["x"])
    beta = _f32(inputs["beta"])
    context = _f32(inputs["context"])
    g = {k: np.asarray(v, dtype=np.float64) for k, v in inputs.items()
         if k not in ("x", "beta", "context")}

    # --- algebraic folds (host, tiny) ---
    embW = g["emb_w"][:, :, 0]            # [64, 3]
    dembW = g["demb_w"][:, :, 0]          # [3, 64]
    M3 = dembW @ embW                     # [3, 3]
    v3 = dembW @ g["emb_b"] + g["demb_b"]
    s3 = M3.sum(axis=1)

    pe = _pe_table().astype(np.float64)   # [N, F]

    c1aug = np.empty((3, F), np.float64)
    c1aug[0:2] = g["c1_w"].T
    c1aug[2] = g["c1_b"]

    # gate weights [C, 29*128]
    wg = np.zeros((C, 29 * 128), np.float32)
    wg[:, 0:1024] = g["c1_gw"].T
    wg[:, 1024:2048] = g["c1_hw"].T
    wg[:, 2048:2560] = g["c3_gw"].T
    wg[:, 2560:3072] = g["c3_hw"].T
    wg[:, 3072:3328] = g["c4_gw"].T
    wg[:, 3328:3584] = g["c4_hw"].T
    wg[:, 3584:3586] = g["cl_gw"].T
    wg[:, 3586:3588] = g["cl_hw"].T
    gbias = np.zeros(29 * 128, np.float32)
    gbias[0:1024] = g["c1_gb"]
    gbias[2048:2560] = g["c3_gb"]
    gbias[3072:3328] = g["c4_gb"]
    gbias[3584:3586] = g["cl_gb"]
    gbias = gbias.reshape(29, 128)
    # [p, c, k, o] = wg[k*128+p, c*128+o]: one prefetched DMA, sliced per c
    wg = np.ascontiguousarray(
        wg.reshape(4, 128, 29, 128).transpose(1, 2, 0, 3))

    # conv weights -> [11, ci, co] tap-major with zero padding
    convt = np.zeros((11, F, F), np.float64)
    convt[5, :, 0:512] = g["conv1_w"][:, :, 0].T
    for t in range(3):
        convt[t + 4, :, 512:768] = g["conv2_w"][:, :, t].T
    for t in range(5):
        convt[t + 3, :, 768:832] = g["conv3_w"][:, :, t].T
    for t in range(7):
        convt[t + 2, :, 832:896] = g["conv4_w"][:, :, t].T
    for t in range(9):
        convt[t + 1, :, 896:960] = g["conv5_w"][:, :, t].T
    for t in range(11):
        convt[t, :, 960:1024] = g["conv6_w"][:, :, t].T

    # pair-grouped conv layout: [delta0 | -1,+1 | -2,+2 | ... | -5,+5]
    convd = np.empty((8, 128, ETOT), np.float32)
    convd[:, :, 0:1024] = convt[5].reshape(8, 128, 1024)
    for k in range(5):
        wk, r0 = WK[k], RUN0K[k]
        convd[:, :, CO_N[k]:CO_N[k] + wk] = \
            convt[5 - (k + 1), :, r0:1024].reshape(8, 128, wk)
        convd[:, :, CO_P[k]:CO_P[k] + wk] = \
            convt[5 + (k + 1), :, r0:1024].reshape(8, 128, wk)

    # positional encoding pushed through the convs (host, exact):
    peT = pe.T                             # [F, N] float64
    pe_conv = np.zeros((F, N), np.float64)
    for d in range(-5, 6):
        a, b2 = max(0, -d), N - max(0, d)
        pe_conv[:, a:b2] += convt[d + 5].T @ peT[:, a + d:b2 + d]
    conv_bias = np.concatenate([g["conv1_b"], g["conv2_b"], g["conv3_b"],
                                g["conv4_b"], g["conv5_b"], g["conv6_b"]])
    c3bias = (g["c3_w"] @ (pe_conv + conv_bias[:, None])
              + g["c3_b"][:, None]).astype(np.float32)   # [C, N]

    # W1T[p, fc, j] = c1aug[j, fc*128+p]
    w1t = _f32(np.ascontiguousarray(
        c1aug.reshape(3, 8, 128).transpose(2, 1, 0)))

    c3wt = _f32(g["c3_w"].T.reshape(8, 128, C).transpose(1, 0, 2))
    c4wt = _f32(g["c4_w"].T.reshape(4, 128, 256).transpose(1, 0, 2))
    c4b = _f32(g["c4_b"].reshape(2, 128))
    clwt = _f32(g["cl_w"].T.reshape(2, 128, PD).transpose(1, 0, 2))
    clb = _f32(g["cl_b"].reshape(PD, 1))

    host = dict(M3=M3, v3=v3, s3=s3)

    shared = dict(wg=_bf16(wg), gbias=gbias, w1t=w1t, convd=_bf16(convd),
                  c3wt=c3wt, c3bias=c3bias, c4wt=c4wt, c4b=c4b,
                  clwt=clwt, clb=clb)

    # xim: block-diag im2col of x (+ bias-mask rows).
    # slot 0 = delta0 (chunk rows duplicated in both halves); slot kappa
    # holds -kappa/+kappa in opposite halves, swapped for odd chunks to
    # match the E-tile pairing.
    xaug = np.empty((3, B, N), np.float32)
    xaug[0:2] = x.transpose(2, 0, 1)
    xaug[2] = 1.0
    in_maps = []
    for k in range(NCORES):
        sl = slice(k * BLOC, (k + 1) * BLOC)
        xim = np.zeros((NBC, 128, 6, FREE), np.float32)
        for bc in range(NBC):
            par = bc % 2
            for sidx in range(6):
                for half in (0, 1):
                    if sidx == 0:
                        dlt = 0
                    else:
                        sgn = -1 if (half == par) else 1
                        dlt = sgn * sidx
                    n0, n1 = max(0, -dlt), min(N, N - dlt)
                    for bh in range(BC):
                        gb = k * BLOC + bc * BC + bh
                        col0 = bh * N
                        for jp in range(3):
                            xim[bc, half * 64 + jp * 16 + bh, sidx,
                                col0 + n0:col0 + n1] = \
                                xaug[jp, gb, n0 + dlt:n1 + dlt]
                        xim[bc, half * 64 + 48 + bh, sidx,
                            col0 + n0:col0 + n1] = 1.0
        m = dict(shared)
        m["ctx"] = np.ascontiguousarray(context[sl])
        m["betav"] = np.ascontiguousarray(beta[sl].reshape(BLOC, 1))
        m["xim"] = _bf16(xim)
        in_maps.append(m)

    return host, in_maps


_LAST_HOST = None


def kernel(**inputs):
    global LAST_RESULTS, _LAST_HOST
    host, in_maps = _host_prep(**inputs)
    _LAST_HOST = host
    trace = bool(int(os.environ.get("KERNEL_TRACE", "0")))
    res = _build_and_run(host, in_maps, trace)
    LAST_RESULTS = res
    out = np.concatenate(
        [res.results[k]["out"].reshape(BLOC, N, PD) for k in range(NCORES)],
        axis=0)
    return out
